# revision 21
# baseline (speedup 1.0000x reference)
"""Trainium2 Bass kernel for the LZD encoder (gnn_message_passing).

Strategy (data parallel over n_seq, 1 sequence per core, 8 cores):
  - h[seq] lives in DRAM as bf16 [4104, 1024]: row 4096 is a garbage sink
    (so every gather/scatter call has a fully static index count) and rows
    4097..4103 guard against any end-of-tensor DMA overreach.
  - Leaf phase: the char-embedding scatter-add is reformulated as a dense
    matmul  h_init = Count @ emb  where Count[pos, char] is computed on the
    host from the (index-only) char event lists.
  - Each of the 12 levels: transposed dma_gather of left/right operand rows
    (X^T tiles, feature-major; split into ramped [1,1,2,4,...]-column groups
    — calls of >512 indices hang the hardware, and small leading groups
    shorten the level-boundary bubble), dense PE matmul against resident W
    (psum [128,1024] spanning 2 banks), bias add on the vector engine, tanh
    on the scalar engine, and per-column dma_scatter_add back into h.  The
    host pre-sorts each level's events into 128-slot columns such that no
    column contains a duplicate destination position, so each per-column
    scatter-add call has unique indices (the hardware scatter-add is not
    atomic across DMA engines).
"""
import numpy as np
import ml_dtypes

from concourse import bass, bacc, tile, mybir
from concourse import bass_utils

BF16 = ml_dtypes.bfloat16
N_SEQ, MAX_LEN, H_DIM, N_CHAR = 8, 4096, 1024, 256
L_LEVELS = 12
SINK = MAX_LEN  # garbage row index in the [4097, 1024] h tensor
N_CORES = 8

_cache = {}
_last_results = None


def _group_sizes(ncol):
    """Gather-group column sizes: [1, 1] ramp then pairs; a group's fused
    left+right gather is 2*size*128 indices and must stay <= 512."""
    sizes = []
    rem = ncol
    for r in (1, 1):
        if rem <= 0:
            break
        sizes.append(1)
        rem -= 1
    while rem > 0:
        t = min(2, rem)
        sizes.append(t)
        rem -= t
    return sizes


# ------------------------------------------------------------------ host prep

def _wrap_idx(idx):
    """int16 index layout for dma_gather/dma_scatter_add: arr[p, s] =
    idx[s*16 + p]; result [16, len/16]."""
    idx = np.asarray(idx, dtype=np.int16)
    assert idx.size % 16 == 0
    return idx.reshape(-1, 16).T


def _place_level(pos, n_cols):
    """Assign each event (with scatter destination pos[i]) a slot in
    [0, n_cols*128) such that no 128-slot column holds two events with the
    same destination. Returns slot array (or None if infeasible)."""
    n = len(pos)
    col_fill = np.zeros(n_cols, dtype=np.int64)
    col_has = [set() for _ in range(n_cols)]
    slots = np.empty(n, dtype=np.int64)
    # place high-multiplicity positions first
    uniq, inv, counts = np.unique(pos, return_inverse=True, return_counts=True)
    order = np.argsort(-counts[inv], kind="stable")
    for i in order:
        p = pos[i]
        best = -1
        for c in range(n_cols):
            if col_fill[c] < 128 and p not in col_has[c]:
                if best < 0 or col_fill[c] < col_fill[best]:
                    best = c
        if best < 0:
            return None
        slots[i] = best * 128 + col_fill[best]
        col_fill[best] += 1
        col_has[best].add(p)
    return slots


def _prep(inputs):
    ci_seq = np.asarray(inputs["char_i_seq"]).astype(np.int64)
    ci_pos = np.asarray(inputs["char_i_pos"]).astype(np.int64)
    c_ids = np.asarray(inputs["char_ids"]).astype(np.int64)
    gi_seq = np.asarray(inputs["grp_i_seq"]).astype(np.int64)
    gi_first = np.asarray(inputs["grp_i_first"]).astype(np.int64)
    gi_second = np.asarray(inputs["grp_i_second"]).astype(np.int64)
    gi_pos = np.asarray(inputs["grp_i_pos"]).astype(np.int64)
    emb = np.asarray(inputs["emb_char"]).astype(np.float32)
    W = np.asarray(inputs["W"]).astype(np.float32)
    b = np.asarray(inputs["b"]).astype(np.float32)

    w_bf = np.ascontiguousarray(W.astype(BF16))
    b_bf = np.ascontiguousarray(np.broadcast_to(b.reshape(1, H_DIM), (128, H_DIM)).astype(np.float32))
    emb_bf = np.ascontiguousarray(emb.astype(BF16))

    # leaf counts: Count[pos, char] per core -> ship transposed [char, pos]
    cnts = []
    for s in range(N_CORES):
        m = ci_seq == s
        cnt = np.zeros((MAX_LEN, N_CHAR), dtype=np.float64)
        np.add.at(cnt, (ci_pos[m], c_ids[m]), 1.0)
        cnts.append(np.ascontiguousarray(cnt.T.astype(BF16)))

    # per-level placement
    per_core_events = [
        [np.nonzero(gi_seq[l] == s)[0] for s in range(N_CORES)]
        for l in range(L_LEVELS)
    ]
    S_list = []
    for l in range(L_LEVELS):
        mx = max(len(per_core_events[l][s]) for s in range(N_CORES))
        S = max(((mx + 127) // 128) * 128, 1152)
        S_list.append(S)

    g1_parts = [[] for _ in range(N_CORES)]
    g2_parts = [[] for _ in range(N_CORES)]
    sc_parts = [[] for _ in range(N_CORES)]
    for l in range(L_LEVELS):
        while True:
            S = S_list[l]
            level_rows = []
            ok = True
            for s in range(N_CORES):
                ev = per_core_events[l][s]
                pos = gi_pos[l][ev]
                slots = _place_level(pos, S // 128)
                if slots is None:  # pragma: no cover - needs multiplicity > ncol
                    S_list[l] = S + 128
                    ok = False
                    break
                g1 = np.zeros(S, dtype=np.int64)
                g2 = np.zeros(S, dtype=np.int64)
                sc = np.full(S, SINK, dtype=np.int64)
                g1[slots] = gi_first[l][ev]
                g2[slots] = gi_second[l][ev]
                sc[slots] = pos
                level_rows.append((g1, g2, sc))
            if ok:
                break
        for s, (g1, g2, sc) in enumerate(level_rows):
            g1_parts[s].append(g1)
            g2_parts[s].append(g2)
            sc_parts[s].append(sc)

    def pack(parts):
        flat = np.concatenate(parts)
        return np.ascontiguousarray(np.tile(_wrap_idx(flat), (8, 1)))

    def fuse(g1_list, g2_list):
        out = []
        for g1, g2 in zip(g1_list, g2_list):
            ncol = len(g1) // 128
            c0 = 0
            for glen in _group_sizes(ncol):
                out.append(g1[c0 * 128 : (c0 + glen) * 128])
                out.append(g2[c0 * 128 : (c0 + glen) * 128])
                c0 += glen
        return out

    in_maps = []
    for s in range(N_CORES):
        in_maps.append(
            {
                "w": w_bf,
                "b": b_bf,
                "emb": emb_bf,
                "cnt": cnts[s],
                "g1": pack(g1_parts[s]),
                "g2": pack(g2_parts[s]),
                "gc": pack(fuse(g1_parts[s], g2_parts[s])),
                "sc": pack(sc_parts[s]),
            }
        )
    return tuple(S_list), in_maps


# -------------------------------------------------------------- device program

def _build(S_list, n_levels=None, leaf=True, no_scatter=False, gcols=4):
    if n_levels is None:
        n_levels = len(S_list)
    tot16 = sum(S_list) // 16
    nc = bacc.Bacc("TRN2", target_bir_lowering=False, debug=False)
    w_in = nc.dram_tensor("w", [2 * H_DIM, H_DIM], mybir.dt.bfloat16,
                          kind="ExternalInput")
    b_in = nc.dram_tensor("b", [128, H_DIM], mybir.dt.float32,
                          kind="ExternalInput")
    emb_in = nc.dram_tensor("emb", [N_CHAR, H_DIM], mybir.dt.bfloat16,
                            kind="ExternalInput")
    cnt_in = nc.dram_tensor("cnt", [N_CHAR, MAX_LEN], mybir.dt.bfloat16,
                            kind="ExternalInput")
    gc_in = nc.dram_tensor("gc", [128, 2 * tot16], mybir.dt.int16,
                           kind="ExternalInput")
    sc_in = nc.dram_tensor("sc", [128, tot16], mybir.dt.int16,
                           kind="ExternalInput")
    h = nc.dram_tensor("h", [MAX_LEN + 8, H_DIM], mybir.dt.bfloat16,
                       kind="ExternalOutput")

    scratch = (nc.dram_tensor("scratch", [2304, H_DIM], mybir.dt.bfloat16)
               if no_scatter else None)
    with tile.TileContext(nc) as tc:
        with (
            tc.tile_pool(name="const", bufs=1) as const,
            tc.tile_pool(name="work", bufs=1) as work,
            tc.tile_pool(name="outp", bufs=4) as outp,
            tc.tile_pool(name="stage", bufs=3) as stage,
            tc.tile_pool(name="ps", bufs=2, space=bass.MemorySpace.PSUM) as ps,
        ):
            # leaf inputs first (the leaf gates everything downstream);
            # cnt split per k-chunk so kc=0 matmuls start at half-load
            emb_sb = const.tile([128, 2, H_DIM], mybir.dt.bfloat16)
            nc.sync.dma_start(emb_sb[:], emb_in.ap().rearrange("(k p) h -> p k h", p=128))
            cnt_sb = const.tile([128, 2, MAX_LEN], mybir.dt.bfloat16)
            nc.sync.dma_start(cnt_sb[:, 0, :], cnt_in.ap()[0:128])
            nc.sync.dma_start(cnt_sb[:, 1, :], cnt_in.ap()[128:256])
            w_sb = const.tile([128, 16, H_DIM], mybir.dt.bfloat16)
            nc.sync.dma_start(w_sb[:], w_in.ap().rearrange("(k p) h -> p k h", p=128))
            bias_sb = const.tile([128, H_DIM], mybir.dt.float32)
            nc.sync.dma_start(bias_sb[:], b_in.ap())
            gc_sb = const.tile([128, 2 * tot16], mybir.dt.int16)
            nc.sync.dma_start(gc_sb[:], gc_in.ap())
            sc_sb = const.tile([128, tot16], mybir.dt.int16)
            nc.sync.dma_start(sc_sb[:], sc_in.ap())
            zrow = const.tile([8, H_DIM], mybir.dt.bfloat16)
            nc.gpsimd.memset(zrow[:], 0.0)
            nc.sync.dma_start(h.ap()[SINK : SINK + 8], zrow[:])

            # ---- leaf phase: h[t*128:(t+1)*128] = Count @ emb
            for t in range(MAX_LEN // 128 if leaf else 0):
                p0 = ps.tile([128, 512], mybir.dt.float32)
                p1 = ps.tile([128, 512], mybir.dt.float32)
                for kc in range(2):
                    lhsT = cnt_sb[:, kc, t * 128 : (t + 1) * 128]
                    nc.tensor.matmul(p0[:], lhsT, emb_sb[:, kc, 0:512],
                                     start=kc == 0, stop=kc == 1)
                    nc.tensor.matmul(p1[:], lhsT, emb_sb[:, kc, 512:1024],
                                     start=kc == 0, stop=kc == 1)
                hst = stage.tile([128, H_DIM], mybir.dt.bfloat16)
                nc.scalar.copy(hst[:, 0:512], p0[:])
                nc.scalar.copy(hst[:, 512:1024], p1[:])
                nc.sync.dma_start(h.ap()[t * 128 : (t + 1) * 128], hst[:])

            def keep_warm(n_mm):
                # dead matmuls that run inside the level-boundary PE-idle
                # window so the HAM clock gate never re-throttles; output
                # is never read (shares the leaf psum tag: no extra banks)
                dmy = ps.tile([128, 512], mybir.dt.float32, tag="p0")
                for _ in range(n_mm):
                    nc.tensor.matmul(dmy[:], w_sb[:, 0, 0:128],
                                     w_sb[:, 1, 0:512], start=True, stop=True)

            if leaf and n_levels:
                keep_warm(64)

            # ---- level phases
            ncols = [S // 128 for S in S_list[:n_levels]]
            offs = [0] * n_levels
            for l in range(1, n_levels):
                offs[l] = offs[l - 1] + ncols[l - 1] * 8

            def alloc_x(ncol_l):
                tiles = []
                for g, glen in enumerate(_group_sizes(ncol_l)):
                    x12 = work.tile([128, 8, 2 * glen * 128],
                                    mybir.dt.bfloat16, tag=f"x_{g}")
                    tiles.append(x12)
                return tiles

            def gather_group(l, g, x12):
                sizes = _group_sizes(ncols[l])
                glen = sizes[g]
                gcol0 = sum(sizes[:g])
                n = 2 * glen * 128
                o = 2 * offs[l] + gcol0 * 16
                nc.gpsimd.dma_gather(
                    x12[:], h.ap(), gc_sb[:, o : o + glen * 16],
                    n, n, H_DIM, transpose=True)

            def col_to_group(sizes):
                m = []
                for g, glen in enumerate(sizes):
                    for jj in range(glen):
                        m.append((g, jj))
                return m

            cur = alloc_x(ncols[0]) if n_levels else None
            if n_levels:
                for g, x12 in enumerate(cur):
                    gather_group(0, g, x12)

            for l in range(n_levels):
                ncol = ncols[l]
                sizes_l = _group_sizes(ncol)
                cmap = col_to_group(sizes_l)
                nxt = alloc_x(ncols[l + 1]) if l + 1 < n_levels else None
                for j in range(ncol):
                    g, jj = cmap[j]
                    glen = sizes_l[g]
                    p = ps.tile([128, H_DIM], mybir.dt.float32)
                    for k in range(16):
                        xt = cur[g]
                        co = jj if k < 8 else glen + jj
                        lhsT = xt[:, k % 8, co * 128 : (co + 1) * 128]
                        first, last = k == 0, k == 15
                        nc.tensor.matmul(p[:, 0:512], lhsT, w_sb[:, k, 0:512],
                                         start=first, stop=last)
                        nc.tensor.matmul(p[:, 512:1024], lhsT,
                                         w_sb[:, k, 512:1024],
                                         start=first, stop=last)
                    nc.vector.tensor_add(p[:], p[:], bias_sb[:])
                    outt = outp.tile([128, 1, H_DIM], mybir.dt.bfloat16, tag="out")
                    nc.scalar.activation(outt[:, 0, :], p[:],
                                         mybir.ActivationFunctionType.Tanh)
                    if no_scatter:
                        nc.sync.dma_start(
                            scratch.ap()[j * 128 : (j + 1) * 128],
                            outt[:, 0, :])
                    else:
                        nc.gpsimd.dma_scatter_add(
                            h.ap(), outt[:],
                            sc_sb[:, offs[l] + j * 8 : offs[l] + (j + 1) * 8],
                            128, 128, H_DIM)
                if nxt is not None:
                    for g, x12 in enumerate(nxt):
                        gather_group(l + 1, g, x12)
                    keep_warm(48)
                cur = nxt

    nc.compile()
    return nc


# -------------------------------------------------------------------- kernel

def kernel(**inputs):
    global _last_results
    S_list, in_maps = _prep(inputs)
    if S_list not in _cache:
        _cache[S_list] = _build(list(S_list))
    nc = _cache[S_list]
    dev_maps = [{k: v for k, v in m.items() if k not in ("g1", "g2")}
                for m in in_maps]
    res = bass_utils.run_bass_kernel_spmd(nc, dev_maps, list(range(N_CORES)))
    _last_results = res
    out = np.zeros((N_SEQ, MAX_LEN, H_DIM), dtype=np.float32)
    for s in range(N_CORES):
        out[s] = res.results[s]["h"][0:MAX_LEN].astype(np.float32)
    out[:, 0, :] = 0.0
    return out
